# revision 15
# baseline (speedup 1.0000x reference)
"""Trainium2 Bass kernel for nn_Encoder_4999341932803: single-layer LSTM.

Reference computation (PyTorch gate order i,f,g,o):
    gates_t = x_t @ Wih.T + h_{t-1} @ Whh.T + (bih + bhh)        # [B, 4H]
    c_t = sig(f)*c_{t-1} + sig(i)*tanh(g)
    h_t = sig(o)*tanh(c_t)
returns (h_T[None], stack(h_t))   shapes: ([1,B,H], [B,T,H])

B=128, T=512, I=128, H=512.

Strategy (data-parallel over batch, 8 cores x 16 rows):
  - Phase 0 (on device): Xg = x @ Wih.T + bias for all timesteps via
    fp32r matmuls, staged to DRAM ("xg" segments).
  - Recurrence: per step, 4 PSUM banks (one per gate, bank order g,i,f,o).
    Each bank: identity-matmul injects Xg_t (start=True), then 4 K-chunk
    fp32r matmuls accumulate h_{t-1} @ Whh.T.
    Activations on ScalarE read PSUM; z = sig(i)*tanh(g) on VectorE.
    z, sig(f), sig(o) are transposed via TensorE (PE transpose) into
    "folded h.T layout" [128, 64] (col = 16k + batch, part = hdim%128),
    where the c/h tail runs cheaply:
        aT=fT*cT; cT=aT+zT; tcT=tanh(cT); hT=oT*tcT
    hT is directly the stationary operand layout for the next step's
    matmuls, and is DMA'd out per step.
Host does only layout transforms (transpose/reorder/fold) + shard/gather.
"""

import numpy as np

B, T, I, H = 128, 512, 128, 512
NCORES = 8
BSH = B // NCORES          # 16 batch rows per core
G4 = 4 * H                 # 2048
KCH = H // 128             # 4 k-chunks of h
NBANK = 4                  # gate banks
SEGS = 8                   # xg dram segments (coarse RAW granularity)
INJECT = "pe"              # "pe" (identity matmuls) or "dve" (engine copies)

# bank order in the reordered weight columns: g, i, f, o
# pytorch row blocks: i=[0,512) f=[512,1024) g=[1024,1536) o=[1536,2048)
_GATE_PERM = np.concatenate([
    np.arange(1024, 1536),   # g  -> bank 0
    np.arange(0, 512),       # i  -> bank 1
    np.arange(512, 1024),    # f  -> bank 2
    np.arange(1536, 2048),   # o  -> bank 3
]).astype(np.int64)


def build_lstm(t_steps=T, b=BSH):
    import concourse.mybir as mybir
    import concourse.tile as tile
    from concourse import bacc

    f32 = mybir.dt.float32
    f32r = mybir.dt.float32r
    AF = mybir.ActivationFunctionType

    mchunks_total = t_steps * b // 128
    segs = min(SEGS, mchunks_total)
    assert t_steps % segs == 0 and mchunks_total % segs == 0
    steps_per_seg = t_steps // segs
    fold = b * KCH             # 64: folded free dim of h.T tiles

    nc = bacc.Bacc("TRN2", target_bir_lowering=False, debug=False)

    xT = nc.dram_tensor("xT", [I, t_steps * b], f32r, kind="ExternalInput")
    wihT = nc.dram_tensor("wihT", [I, G4], f32r, kind="ExternalInput")
    whhT = nc.dram_tensor("whhT", [H, G4], f32r, kind="ExternalInput")
    biasT = nc.dram_tensor("biasT", [1, G4], f32r, kind="ExternalInput")
    h0T = nc.dram_tensor("h0T", [128, fold], f32r, kind="ExternalInput")
    c0T = nc.dram_tensor("c0T", [128, fold], f32, kind="ExternalInput")
    hsT = nc.dram_tensor("hsT", [t_steps, 128, fold], f32r, kind="ExternalOutput")
    id16T = nc.dram_tensor("id16T", [b, b], f32r, kind="ExternalInput")
    id16FT = nc.dram_tensor("id16FT", [b, b], f32, kind="ExternalInput")
    onesT = nc.dram_tensor("onesT", [1, 128], f32r, kind="ExternalInput")

    mchunks = mchunks_total                # precompute M chunks
    mch_per_seg = mchunks // segs

    with tile.TileContext(nc) as tc:
        import contextlib
        ctx = contextlib.ExitStack()
        with ctx:
            consts = ctx.enter_context(tc.tile_pool(name="consts", bufs=1))
            dram = ctx.enter_context(tc.tile_pool(name="dram", bufs=1, space="DRAM"))
            pcpool = ctx.enter_context(tc.tile_pool(name="pc", bufs=3))
            xgpool = ctx.enter_context(tc.tile_pool(name="xg", bufs=6))
            work = ctx.enter_context(tc.tile_pool(name="work", bufs=2))
            state = ctx.enter_context(tc.tile_pool(name="state", bufs=2))
            pg = ctx.enter_context(tc.tile_pool(name="pgates", bufs=1, space="PSUM"))
            ptr = ctx.enter_context(tc.tile_pool(name="ptrans", bufs=1, space="PSUM"))
            ppc = ctx.enter_context(tc.tile_pool(name="ppc", bufs=2, space="PSUM"))

            # ---- constants / weights resident in SBUF ----
            id16 = consts.tile([b, b], f32r, tag="id16")
            nc.sync.dma_start(out=id16, in_=id16T[:, :])
            id16f = consts.tile([b, b], f32, tag="id16f")
            nc.sync.dma_start(out=id16f, in_=id16FT[:, :])
            import concourse.bass as bass
            from concourse.tile import add_dep_helper
            ones1 = consts.tile([1, 128], f32r, tag="ones1")
            nc.sync.dma_start(out=ones1, in_=onesT[:, :])
            bias16 = consts.tile([b, G4], f32r, tag="bias16")
            bias_bcast = bass.AP(tensor=biasT, offset=0,
                                 ap=[[0, b], [1, G4]])
            nc.gpsimd.dma_start(out=bias16, in_=bias_bcast)
            wih_sb = consts.tile([128, G4], f32r, tag="wih_sb")
            nc.sync.dma_start(out=wih_sb, in_=wihT[:, :])
            whh_sb = []
            for k in range(KCH):
                wk = consts.tile([128, G4], f32r, tag=f"whh_sb{k}", name=f"whh_sb{k}")
                nc.sync.dma_start(out=wk, in_=whhT[128 * k:128 * (k + 1), :])
                whh_sb.append(wk)

            # xg staging in DRAM, segmented for coarse precompute->recurrence overlap
            xg_segs = [
                dram.tile([mch_per_seg * 128, G4], f32r, tag=f"xgseg{s}",
                          name=f"xgseg{s}")
                for s in range(segs)
            ]

            # ---- phase 0: Xg = x @ Wih.T + bias  (fp32r matmuls) ----
            for m in range(mchunks):
                seg = m // mch_per_seg
                mrow = (m % mch_per_seg) * 128
                xch = pcpool.tile([128, 128], f32r, tag="xch")
                nc.sync.dma_start(out=xch, in_=xT[:, 128 * m:128 * (m + 1)])
                pps = ppc.tile([128, 512], f32, tag="ppc")
                out_sb = pcpool.tile([128, 512], f32r, tag="pc_out")
                for n in range(NBANK):
                    nsl = slice(512 * n, 512 * (n + 1))
                    nc.tensor.matmul(pps[:, :], xch,
                                     wih_sb[:, nsl],
                                     start=True, stop=False)
                    nc.tensor.matmul(pps[:, :], ones1,
                                     bias16[0:1, nsl],
                                     start=False, stop=True)
                    # copy PSUM -> SBUF (alternate engines to split load)
                    if n % 2 == 0:
                        nc.scalar.copy(out_sb[:, :], pps[:, :])
                    else:
                        nc.vector.tensor_copy(out_sb[:, :], pps[:, :])
                    nc.sync.dma_start(
                        out=xg_segs[seg][mrow:mrow + 128, nsl], in_=out_sb)
                    if n != NBANK - 1:
                        pps = ppc.tile([128, 512], f32, tag="ppc")
                        out_sb = pcpool.tile([128, 512], f32r, tag="pc_out")

            if INJECT != "pe":
                # ---- prime the gate banks' has_written bits (once) ----
                # After this every element's has_written stays set, so the
                # per-step Whh matmuls (start=False, skip_group_check)
                # accumulate onto whatever was pre-written into the bank.
                for n in range(NBANK):
                    pbank = pg.tile([b, 512], f32, tag=f"bank{n}",
                                    name=f"primer{n}")
                    nc.tensor.matmul(pbank[:, :], id16, bias16[:, 0:512],
                                     start=True, stop=True)

            # ---- initial state ----
            hT_prev = state.tile([128, fold], f32r, tag="hT")
            nc.sync.dma_start(out=hT_prev, in_=h0T[:, :])
            cT_prev = state.tile([128, fold], f32, tag="cT")
            nc.sync.dma_start(out=cT_prev, in_=c0T[:, :])

            # ---- recurrence ----
            prev = None
            for t in range(t_steps):
                seg = t // steps_per_seg
                r0 = (t % steps_per_seg) * b
                xg_t = xgpool.tile([b, G4], f32r, tag="xg_t")
                nc.sync.dma_start(out=xg_t, in_=xg_segs[seg][r0:r0 + b, :])

                # gate banks: order g,i,f,o.
                # Xg(+bias, folded in at precompute) is injected into each
                # PSUM bank, then the Whh matmuls accumulate on top.
                # INJECT="pe": 4 identity matmuls (start=True) — keeps the PE
                # densely fed (HAM stays warm) at ~0.93us/step of PE time.
                # INJECT="dve": off-PE copies with explicit queue placement.
                banks = []
                for n in range(NBANK):
                    pbank = pg.tile([b, 512], f32, tag=f"bank{n}",
                                    name=f"bank{n}_{t}")
                    banks.append(pbank)
                if INJECT == "pe":
                    for n in range(NBANK):
                        nsl = slice(512 * n, 512 * (n + 1))
                        nc.tensor.matmul(banks[n][:, :], id16, xg_t[:, nsl],
                                         start=True, stop=False)
                    for n in range(NBANK):
                        nsl = slice(512 * n, 512 * (n + 1))
                        for k in range(KCH):
                            nc.tensor.matmul(
                                banks[n][:, :],
                                hT_prev[:, b * k:b * (k + 1)],
                                whh_sb[k][:, nsl],
                                start=False, stop=(k == KCH - 1))
                else:
                    inj = []
                    for n in range(NBANK):
                        nsl = slice(512 * n, 512 * (n + 1))
                        if n < 2:
                            ins = nc.scalar.copy(banks[n][:, :], xg_t[:, nsl])
                        else:
                            ins = nc.vector.tensor_copy(banks[n][:, :],
                                                        xg_t[:, nsl])
                        inj.append(ins)
                    if prev is not None:
                        add_dep_helper(inj[0].ins, prev["tcT"].ins, False,
                                       "inject after prev tanh_cT")
                        add_dep_helper(inj[1].ins, inj[0].ins, False,
                                       "inject order")
                        add_dep_helper(inj[2].ins, prev["z"].ins, False,
                                       "inject f after prev z")
                        add_dep_helper(prev["aT"].ins, inj[2].ins, False,
                                       "prev aT after inject f")
                        add_dep_helper(inj[3].ins, prev["hT"].ins, False,
                                       "inject o after prev hT")
                    for n in range(NBANK):
                        nsl = slice(512 * n, 512 * (n + 1))
                        for k in range(KCH):
                            nc.tensor.matmul(
                                banks[n][:, :],
                                hT_prev[:, b * k:b * (k + 1)],
                                whh_sb[k][:, nsl],
                                start=False, stop=False,
                                skip_group_check=True)

                g_sb = work.tile([b, 512], f32, tag="g_sb")
                nc.scalar.activation(g_sb[:, :], banks[0][:, :], AF.Tanh)
                i_sb = work.tile([b, 512], f32, tag="i_sb")
                nc.scalar.activation(i_sb[:, :], banks[1][:, :], AF.Sigmoid)
                f_sb = work.tile([b, 512], f32, tag="f_sb")
                nc.scalar.activation(f_sb[:, :], banks[2][:, :], AF.Sigmoid)
                o_sb = work.tile([b, 512], f32, tag="o_sb")
                nc.scalar.activation(o_sb[:, :], banks[3][:, :], AF.Sigmoid)

                z_sb = work.tile([b, 512], f32, tag="z_sb")
                i_z = nc.vector.tensor_mul(z_sb[:, :], i_sb[:, :], g_sb[:, :])

                # transpose z, f, o into folded h.T layout via PE
                pzf = ptr.tile([128, 2 * fold], f32, tag="pzf", name=f"pzf_{t}")
                po = ptr.tile([128, fold], f32, tag="po", name=f"po_{t}")
                for k in range(KCH):
                    nc.tensor.transpose(pzf[:, b * k:b * (k + 1)],
                                        z_sb[:, 128 * k:128 * (k + 1)], id16f)
                for k in range(KCH):
                    nc.tensor.transpose(pzf[:, fold + b * k:fold + b * (k + 1)],
                                        f_sb[:, 128 * k:128 * (k + 1)], id16f)
                for k in range(KCH):
                    nc.tensor.transpose(po[:, b * k:b * (k + 1)],
                                        o_sb[:, 128 * k:128 * (k + 1)], id16f)

                # c/h tail reads the transposed gates straight from PSUM
                # (one PSUM operand per op), skipping PSUM->SBUF copies.
                aT = work.tile([128, fold], f32, tag="aT")
                i_aT = nc.vector.tensor_mul(aT[:, :], pzf[:, fold:2 * fold],
                                            cT_prev[:, :])
                cT = state.tile([128, fold], f32, tag="cT", name=f"cT_{t}")
                nc.vector.tensor_add(cT[:, :], aT[:, :], pzf[:, 0:fold])
                tcT = work.tile([128, fold], f32, tag="tcT")
                i_tcT = nc.scalar.activation(tcT[:, :], cT[:, :], AF.Tanh)
                hT_t = state.tile([128, fold], f32r, tag="hT", name=f"hT_{t}")
                i_hT = nc.vector.tensor_mul(hT_t[:, :], po[:, :], tcT[:, :])

                nc.sync.dma_start(out=hsT[t, :, :], in_=hT_t)

                prev = {"z": i_z, "aT": i_aT, "tcT": i_tcT, "hT": i_hT}
                hT_prev, cT_prev = hT_t, cT

    nc.compile()
    return nc


# ---------------- host-side marshalling ----------------

def _fold_state(s):
    # s: [b, H] -> folded transposed [128, b*KCH], col = k*b + batch
    b = s.shape[0]
    return np.ascontiguousarray(
        s.reshape(b, KCH, 128).transpose(2, 1, 0).reshape(128, KCH * b))


def _unfold_hs(hsT_out, b):
    # hsT_out: [T, 128, KCH*b] -> hs [b, T, H], h[bt, t, 128k+p] = hsT[t, p, k*b+bt]
    t_steps = hsT_out.shape[0]
    return np.ascontiguousarray(
        hsT_out.reshape(t_steps, 128, KCH, b).transpose(3, 0, 2, 1)
        .reshape(b, t_steps, KCH * 128))


def prepare_inputs(input_data, Wih, Whh, bih, bhh, h0, c0, t_steps=T):
    """Builds the 8 per-core input maps (pure layout transforms)."""
    x = np.asarray(input_data, dtype=np.float32)
    Wih = np.asarray(Wih, dtype=np.float32)
    Whh = np.asarray(Whh, dtype=np.float32)
    bias = (np.asarray(bih, dtype=np.float32)
            + np.asarray(bhh, dtype=np.float32))
    h0 = np.asarray(h0, dtype=np.float32)
    c0 = np.asarray(c0, dtype=np.float32)

    wihT = np.ascontiguousarray(Wih[_GATE_PERM].T)          # [I, 4H]
    whhT = np.ascontiguousarray(Whh[_GATE_PERM].T)          # [H, 4H]
    biasT = np.ascontiguousarray(bias[_GATE_PERM][None])    # [1, 4H]

    in_maps = []
    for m in range(NCORES):
        sl = slice(BSH * m, BSH * (m + 1))
        xs = x[sl, :t_steps]                                 # [b, T, I]
        xT = np.ascontiguousarray(
            xs.transpose(2, 1, 0).reshape(I, t_steps * BSH))
        in_maps.append({
            "xT": xT,
            "wihT": wihT,
            "whhT": whhT,
            "biasT": biasT,
            "h0T": _fold_state(h0[sl]),
            "c0T": _fold_state(c0[sl]),
            "id16T": np.eye(BSH, dtype=np.float32),
            "id16FT": np.eye(BSH, dtype=np.float32),
            "onesT": np.ones((1, 128), dtype=np.float32),
        })
    return in_maps


def gather_outputs(results, t_steps=T):
    hs = np.empty((B, t_steps, H), dtype=np.float32)
    for m, r in enumerate(results):
        hs[BSH * m:BSH * (m + 1)] = _unfold_hs(np.asarray(r["hsT"]), BSH)
    last = hs[:, -1, :][None]
    return last, hs


_BUILT = {}


def kernel(input_data, Wih, Whh, bih, bhh, h0, c0):
    from concourse.bass_utils import run_bass_kernel_spmd

    if T not in _BUILT:
        _BUILT[T] = build_lstm(T)
    nc = _BUILT[T]
    in_maps = prepare_inputs(input_data, Wih, Whh, bih, bhh, h0, c0)
    res = run_bass_kernel_spmd(nc, in_maps, core_ids=list(range(NCORES)))
    return gather_outputs(res.results)


# ---------------- CoreSim logic check (dev only) ----------------

def _sim_check(t_steps=16):
    import jax
    jax.config.update("jax_platforms", "cpu")
    from concourse.bass_interp import CoreSim

    rng = np.random.default_rng(0)
    s = 1.0 / np.sqrt(H)
    x = rng.standard_normal((B, t_steps, I), dtype=np.float32)
    Wih = rng.uniform(-s, s, (G4, I)).astype(np.float32)
    Whh = rng.uniform(-s, s, (G4, H)).astype(np.float32)
    bih = rng.uniform(-s, s, G4).astype(np.float32)
    bhh = rng.uniform(-s, s, G4).astype(np.float32)
    h0 = (rng.standard_normal((B, H)) * 0.04).astype(np.float32)
    c0 = (rng.standard_normal((B, H)) * 0.04).astype(np.float32)

    # numpy reference
    bb = bih + bhh
    h, c = h0.copy(), c0.copy()
    hs_ref = np.empty((B, t_steps, H), np.float32)
    sig = lambda v: 1.0 / (1.0 + np.exp(-v))
    for t in range(t_steps):
        gates = x[:, t] @ Wih.T + h @ Whh.T + bb
        i, f, g, o = np.split(gates, 4, axis=1)
        c = sig(f) * c + sig(i) * np.tanh(g)
        h = sig(o) * np.tanh(c)
        hs_ref[:, t] = h

    nc = build_lstm(t_steps)
    in_maps = prepare_inputs(x, Wih, Whh, bih, bhh, h0, c0, t_steps)

    sim = CoreSim(nc)
    for name, val in in_maps[0].items():
        sim.tensor(name)[:] = val
    sim.simulate()
    hsT_out = np.asarray(sim.tensor("hsT"))
    hs = _unfold_hs(hsT_out, BSH)
    ref = hs_ref[:BSH]
    err = np.abs(hs - ref).max()
    rel = err / max(np.abs(ref).max(), 1e-9)
    print(f"sim t_steps={t_steps}: max abs err={err:.3e} rel={rel:.3e}")
    assert rel < 1e-4, "sim mismatch"
    print("SIM PASS")


if __name__ == "__main__":
    import sys
    _sim_check(int(sys.argv[1]) if len(sys.argv) > 1 else 16)


# revision 17
# speedup vs baseline: 3.7767x; 3.7767x over previous
"""Trainium2 Bass kernel for nn_Encoder_4999341932803: single-layer LSTM.

Reference computation (PyTorch gate order i,f,g,o):
    gates_t = x_t @ Wih.T + h_{t-1} @ Whh.T + (bih + bhh)        # [B, 4H]
    c_t = sig(f)*c_{t-1} + sig(i)*tanh(g)
    h_t = sig(o)*tanh(c_t)
returns (h_T[None], stack(h_t))   shapes: ([1,B,H], [B,T,H])

B=128, T=512, I=128, H=512.

Strategy (data-parallel over batch, 8 cores x 16 rows):
  - Phase 0 (on device): Xg = x @ Wih.T + bias for all timesteps via
    fp32r matmuls, staged to DRAM ("xg" segments).
  - Recurrence: per step, 4 PSUM banks (one per gate, bank order g,i,f,o).
    Each bank: identity-matmul injects Xg_t (start=True), then 4 K-chunk
    fp32r matmuls accumulate h_{t-1} @ Whh.T.
    Activations on ScalarE read PSUM; z = sig(i)*tanh(g) on VectorE.
    z, sig(f), sig(o) are transposed via TensorE (PE transpose) into
    "folded h.T layout" [128, 64] (col = 16k + batch, part = hdim%128),
    where the c/h tail runs cheaply:
        aT=fT*cT; cT=aT+zT; tcT=tanh(cT); hT=oT*tcT
    hT is directly the stationary operand layout for the next step's
    matmuls, and is DMA'd out per step.
Host does only layout transforms (transpose/reorder/fold) + shard/gather.
"""

import numpy as np

B, T, I, H = 128, 512, 128, 512
NCORES = 8
BSH = B // NCORES          # 16 batch rows per core
G4 = 4 * H                 # 2048
KCH = H // 128             # 4 k-chunks of h
NBANK = 4                  # gate banks
SEGS = 8                   # xg dram segments (coarse RAW granularity)
INJECT = "dve"              # "pe" (identity matmuls) or "dve" (engine copies)

# bank order in the reordered weight columns: g, i, f, o
# pytorch row blocks: i=[0,512) f=[512,1024) g=[1024,1536) o=[1536,2048)
_GATE_PERM = np.concatenate([
    np.arange(1024, 1536),   # g  -> bank 0
    np.arange(0, 512),       # i  -> bank 1
    np.arange(512, 1024),    # f  -> bank 2
    np.arange(1536, 2048),   # o  -> bank 3
]).astype(np.int64)


def build_lstm(t_steps=T, b=BSH):
    import concourse.mybir as mybir
    import concourse.tile as tile
    from concourse import bacc

    f32 = mybir.dt.float32
    f32r = mybir.dt.float32r
    AF = mybir.ActivationFunctionType

    mchunks_total = t_steps * b // 128
    segs = min(SEGS, mchunks_total)
    assert t_steps % segs == 0 and mchunks_total % segs == 0
    steps_per_seg = t_steps // segs
    fold = b * KCH             # 64: folded free dim of h.T tiles

    nc = bacc.Bacc("TRN2", target_bir_lowering=False, debug=False)

    xT = nc.dram_tensor("xT", [I, t_steps * b], f32r, kind="ExternalInput")
    wihT = nc.dram_tensor("wihT", [I, G4], f32r, kind="ExternalInput")
    whhT = nc.dram_tensor("whhT", [H, G4], f32r, kind="ExternalInput")
    biasT = nc.dram_tensor("biasT", [1, G4], f32r, kind="ExternalInput")
    h0T = nc.dram_tensor("h0T", [128, fold], f32r, kind="ExternalInput")
    c0T = nc.dram_tensor("c0T", [128, fold], f32, kind="ExternalInput")
    hsT = nc.dram_tensor("hsT", [t_steps, 128, fold], f32r, kind="ExternalOutput")
    id16T = nc.dram_tensor("id16T", [b, b], f32r, kind="ExternalInput")
    id16FT = nc.dram_tensor("id16FT", [b, b], f32, kind="ExternalInput")
    onesT = nc.dram_tensor("onesT", [1, 128], f32r, kind="ExternalInput")

    mchunks = mchunks_total                # precompute M chunks
    mch_per_seg = mchunks // segs

    with tile.TileContext(nc) as tc:
        import contextlib
        ctx = contextlib.ExitStack()
        with ctx:
            consts = ctx.enter_context(tc.tile_pool(name="consts", bufs=1))
            dram = ctx.enter_context(tc.tile_pool(name="dram", bufs=1, space="DRAM"))
            pcpool = ctx.enter_context(tc.tile_pool(name="pc", bufs=3))
            xgpool = ctx.enter_context(tc.tile_pool(name="xg", bufs=6))
            work = ctx.enter_context(tc.tile_pool(name="work", bufs=2))
            state = ctx.enter_context(tc.tile_pool(name="state", bufs=2))
            pg = ctx.enter_context(tc.tile_pool(name="pgates", bufs=1, space="PSUM"))
            ptr = ctx.enter_context(tc.tile_pool(name="ptrans", bufs=1, space="PSUM"))
            ppc = ctx.enter_context(tc.tile_pool(name="ppc", bufs=2, space="PSUM"))

            # ---- constants / weights resident in SBUF ----
            id16 = consts.tile([b, b], f32r, tag="id16")
            nc.sync.dma_start(out=id16, in_=id16T[:, :])
            id16f = consts.tile([b, b], f32, tag="id16f")
            nc.sync.dma_start(out=id16f, in_=id16FT[:, :])
            import concourse.bass as bass
            from concourse.tile import add_dep_helper
            ones1 = consts.tile([1, 128], f32r, tag="ones1")
            nc.sync.dma_start(out=ones1, in_=onesT[:, :])
            bias16 = consts.tile([b, G4], f32r, tag="bias16")
            bias_bcast = bass.AP(tensor=biasT, offset=0,
                                 ap=[[0, b], [1, G4]])
            nc.gpsimd.dma_start(out=bias16, in_=bias_bcast)
            wih_sb = consts.tile([128, G4], f32r, tag="wih_sb")
            nc.sync.dma_start(out=wih_sb, in_=wihT[:, :])
            whh_sb = []
            for k in range(KCH):
                wk = consts.tile([128, G4], f32r, tag=f"whh_sb{k}", name=f"whh_sb{k}")
                nc.sync.dma_start(out=wk, in_=whhT[128 * k:128 * (k + 1), :])
                whh_sb.append(wk)

            # xg staging in DRAM, segmented for coarse precompute->recurrence overlap
            xg_segs = [
                dram.tile([mch_per_seg * 128, G4], f32r, tag=f"xgseg{s}",
                          name=f"xgseg{s}")
                for s in range(segs)
            ]

            # ---- phase 0: Xg = x @ Wih.T + bias  (fp32r matmuls) ----
            for m in range(mchunks):
                seg = m // mch_per_seg
                mrow = (m % mch_per_seg) * 128
                xch = pcpool.tile([128, 128], f32r, tag="xch")
                nc.sync.dma_start(out=xch, in_=xT[:, 128 * m:128 * (m + 1)])
                pps = ppc.tile([128, 512], f32, tag="ppc")
                out_sb = pcpool.tile([128, 512], f32r, tag="pc_out")
                for n in range(NBANK):
                    nsl = slice(512 * n, 512 * (n + 1))
                    nc.tensor.matmul(pps[:, :], xch,
                                     wih_sb[:, nsl],
                                     start=True, stop=False)
                    nc.tensor.matmul(pps[:, :], ones1,
                                     bias16[0:1, nsl],
                                     start=False, stop=True)
                    # copy PSUM -> SBUF (alternate engines to split load)
                    if n % 2 == 0:
                        nc.scalar.copy(out_sb[:, :], pps[:, :])
                    else:
                        nc.vector.tensor_copy(out_sb[:, :], pps[:, :])
                    nc.sync.dma_start(
                        out=xg_segs[seg][mrow:mrow + 128, nsl], in_=out_sb)
                    if n != NBANK - 1:
                        pps = ppc.tile([128, 512], f32, tag="ppc")
                        out_sb = pcpool.tile([128, 512], f32r, tag="pc_out")

            if INJECT != "pe":
                # ---- prime the gate banks' has_written bits (once) ----
                # After this every element's has_written stays set, so the
                # per-step Whh matmuls (start=False, skip_group_check)
                # accumulate onto whatever was pre-written into the bank.
                for n in range(NBANK):
                    pbank = pg.tile([b, 512], f32, tag=f"bank{n}",
                                    name=f"primer{n}")
                    nc.tensor.matmul(pbank[:, :], id16, bias16[:, 0:512],
                                     start=True, stop=True)

            # ---- initial state ----
            hT_prev = state.tile([128, fold], f32r, tag="hT")
            nc.sync.dma_start(out=hT_prev, in_=h0T[:, :])
            cT_prev = state.tile([128, fold], f32, tag="cT")
            nc.sync.dma_start(out=cT_prev, in_=c0T[:, :])

            # ---- recurrence ----
            prev = None
            filler_insts = []
            for t in range(t_steps):
                seg = t // steps_per_seg
                r0 = (t % steps_per_seg) * b
                xg_t = xgpool.tile([b, G4], f32r, tag="xg_t")
                nc.sync.dma_start(out=xg_t, in_=xg_segs[seg][r0:r0 + b, :])

                # gate banks: order g,i,f,o.
                # Xg(+bias, folded in at precompute) is injected into each
                # PSUM bank, then the Whh matmuls accumulate on top.
                # INJECT="pe": 4 identity matmuls (start=True) — keeps the PE
                # densely fed (HAM stays warm) at ~0.93us/step of PE time.
                # INJECT="dve": off-PE copies with explicit queue placement.
                banks = []
                for n in range(NBANK):
                    pbank = pg.tile([b, 512], f32, tag=f"bank{n}",
                                    name=f"bank{n}_{t}")
                    banks.append(pbank)
                if INJECT == "pe":
                    for n in range(NBANK):
                        nsl = slice(512 * n, 512 * (n + 1))
                        nc.tensor.matmul(banks[n][:, :], id16, xg_t[:, nsl],
                                         start=True, stop=False)
                    for n in range(NBANK):
                        nsl = slice(512 * n, 512 * (n + 1))
                        for k in range(KCH):
                            nc.tensor.matmul(
                                banks[n][:, :],
                                hT_prev[:, b * k:b * (k + 1)],
                                whh_sb[k][:, nsl],
                                start=False, stop=(k == KCH - 1))
                else:
                    # all four injections on VectorE; their DVE queue slots
                    # are pinned relative to the previous step's tail:
                    #   [inj_g, z(prev), inj_i, aT, cT, hT, inj_f, inj_o]
                    inj = []
                    for n in range(NBANK):
                        nsl = slice(512 * n, 512 * (n + 1))
                        ins = nc.vector.tensor_copy(banks[n][:, :],
                                                    xg_t[:, nsl])
                        inj.append(ins)
                    if prev is not None:
                        add_dep_helper(prev["z"].ins, inj[0].ins, False,
                                       "prev z after inj_g")
                        add_dep_helper(inj[1].ins, prev["z"].ins, False,
                                       "inj_i after prev z")
                        add_dep_helper(prev["aT"].ins, inj[1].ins, False,
                                       "prev aT after inj_i")
                        add_dep_helper(inj[2].ins, prev["hT"].ins, False,
                                       "inj_f after prev hT")
                        add_dep_helper(inj[3].ins, inj[2].ins, False,
                                       "inj_o after inj_f")
                    for n in range(NBANK):
                        nsl = slice(512 * n, 512 * (n + 1))
                        for k in range(KCH):
                            nc.tensor.matmul(
                                banks[n][:, :],
                                hT_prev[:, b * k:b * (k + 1)],
                                whh_sb[k][:, nsl],
                                start=False, stop=False,
                                skip_group_check=True)
                    # PE warmth filler: one throwaway N=512 matmul into the
                    # (otherwise idle) precompute psum slot per step, ordered
                    # after this step's transposes so it lands in the PE idle
                    # window while the tail drains.
                    fill_ps = ppc.tile([128, 512], f32, tag="ppc",
                                       name=f"fill_{t}")
                    i_fill = nc.tensor.matmul(fill_ps[0:16, :], id16,
                                              bias16[:, 0:512],
                                              start=True, stop=True)
                    filler_insts.append(i_fill)

                g_sb = work.tile([b, 512], f32, tag="g_sb")
                nc.scalar.activation(g_sb[:, :], banks[0][:, :], AF.Tanh)
                i_sb = work.tile([b, 512], f32, tag="i_sb")
                nc.scalar.activation(i_sb[:, :], banks[1][:, :], AF.Sigmoid)
                f_sb = work.tile([b, 512], f32, tag="f_sb")
                nc.scalar.activation(f_sb[:, :], banks[2][:, :], AF.Sigmoid)
                o_sb = work.tile([b, 512], f32, tag="o_sb")
                nc.scalar.activation(o_sb[:, :], banks[3][:, :], AF.Sigmoid)

                z_sb = work.tile([b, 512], f32, tag="z_sb")
                i_z = nc.vector.tensor_mul(z_sb[:, :], i_sb[:, :], g_sb[:, :])

                # transpose z, f, o into folded h.T layout via PE
                pzf = ptr.tile([128, 2 * fold], f32, tag="pzf", name=f"pzf_{t}")
                po = ptr.tile([128, fold], f32, tag="po", name=f"po_{t}")
                for k in range(KCH):
                    nc.tensor.transpose(pzf[:, b * k:b * (k + 1)],
                                        z_sb[:, 128 * k:128 * (k + 1)], id16f)
                for k in range(KCH):
                    nc.tensor.transpose(pzf[:, fold + b * k:fold + b * (k + 1)],
                                        f_sb[:, 128 * k:128 * (k + 1)], id16f)
                last_tr = None
                for k in range(KCH):
                    last_tr = nc.tensor.transpose(
                        po[:, b * k:b * (k + 1)],
                        o_sb[:, 128 * k:128 * (k + 1)], id16f)
                if INJECT != "pe" and filler_insts:
                    add_dep_helper(filler_insts[-1].ins, last_tr.ins, False,
                                   "filler after o-transposes")

                # c/h tail reads the transposed gates straight from PSUM
                # (one PSUM operand per op), skipping PSUM->SBUF copies.
                aT = work.tile([128, fold], f32, tag="aT")
                i_aT = nc.vector.tensor_mul(aT[:, :], pzf[:, fold:2 * fold],
                                            cT_prev[:, :])
                cT = state.tile([128, fold], f32, tag="cT", name=f"cT_{t}")
                nc.vector.tensor_add(cT[:, :], aT[:, :], pzf[:, 0:fold])
                tcT = work.tile([128, fold], f32, tag="tcT")
                i_tcT = nc.scalar.activation(tcT[:, :], cT[:, :], AF.Tanh)
                hT_t = state.tile([128, fold], f32r, tag="hT", name=f"hT_{t}")
                i_hT = nc.vector.tensor_mul(hT_t[:, :], po[:, :], tcT[:, :])

                nc.sync.dma_start(out=hsT[t, :, :], in_=hT_t)

                prev = {"z": i_z, "aT": i_aT, "tcT": i_tcT, "hT": i_hT}
                hT_prev, cT_prev = hT_t, cT

    nc.compile()
    return nc


# ---------------- host-side marshalling ----------------

def _fold_state(s):
    # s: [b, H] -> folded transposed [128, b*KCH], col = k*b + batch
    b = s.shape[0]
    return np.ascontiguousarray(
        s.reshape(b, KCH, 128).transpose(2, 1, 0).reshape(128, KCH * b))


def _unfold_hs(hsT_out, b):
    # hsT_out: [T, 128, KCH*b] -> hs [b, T, H], h[bt, t, 128k+p] = hsT[t, p, k*b+bt]
    t_steps = hsT_out.shape[0]
    return np.ascontiguousarray(
        hsT_out.reshape(t_steps, 128, KCH, b).transpose(3, 0, 2, 1)
        .reshape(b, t_steps, KCH * 128))


def prepare_inputs(input_data, Wih, Whh, bih, bhh, h0, c0, t_steps=T):
    """Builds the 8 per-core input maps (pure layout transforms)."""
    x = np.asarray(input_data, dtype=np.float32)
    Wih = np.asarray(Wih, dtype=np.float32)
    Whh = np.asarray(Whh, dtype=np.float32)
    bias = (np.asarray(bih, dtype=np.float32)
            + np.asarray(bhh, dtype=np.float32))
    h0 = np.asarray(h0, dtype=np.float32)
    c0 = np.asarray(c0, dtype=np.float32)

    wihT = np.ascontiguousarray(Wih[_GATE_PERM].T)          # [I, 4H]
    whhT = np.ascontiguousarray(Whh[_GATE_PERM].T)          # [H, 4H]
    biasT = np.ascontiguousarray(bias[_GATE_PERM][None])    # [1, 4H]

    in_maps = []
    for m in range(NCORES):
        sl = slice(BSH * m, BSH * (m + 1))
        xs = x[sl, :t_steps]                                 # [b, T, I]
        xT = np.ascontiguousarray(
            xs.transpose(2, 1, 0).reshape(I, t_steps * BSH))
        in_maps.append({
            "xT": xT,
            "wihT": wihT,
            "whhT": whhT,
            "biasT": biasT,
            "h0T": _fold_state(h0[sl]),
            "c0T": _fold_state(c0[sl]),
            "id16T": np.eye(BSH, dtype=np.float32),
            "id16FT": np.eye(BSH, dtype=np.float32),
            "onesT": np.ones((1, 128), dtype=np.float32),
        })
    return in_maps


def gather_outputs(results, t_steps=T):
    hs = np.empty((B, t_steps, H), dtype=np.float32)
    for m, r in enumerate(results):
        hs[BSH * m:BSH * (m + 1)] = _unfold_hs(np.asarray(r["hsT"]), BSH)
    last = hs[:, -1, :][None]
    return last, hs


_BUILT = {}


def kernel(input_data, Wih, Whh, bih, bhh, h0, c0):
    from concourse.bass_utils import run_bass_kernel_spmd

    if T not in _BUILT:
        _BUILT[T] = build_lstm(T)
    nc = _BUILT[T]
    in_maps = prepare_inputs(input_data, Wih, Whh, bih, bhh, h0, c0)
    res = run_bass_kernel_spmd(nc, in_maps, core_ids=list(range(NCORES)))
    return gather_outputs(res.results)


# ---------------- CoreSim logic check (dev only) ----------------

def _sim_check(t_steps=16):
    import jax
    jax.config.update("jax_platforms", "cpu")
    from concourse.bass_interp import CoreSim

    rng = np.random.default_rng(0)
    s = 1.0 / np.sqrt(H)
    x = rng.standard_normal((B, t_steps, I), dtype=np.float32)
    Wih = rng.uniform(-s, s, (G4, I)).astype(np.float32)
    Whh = rng.uniform(-s, s, (G4, H)).astype(np.float32)
    bih = rng.uniform(-s, s, G4).astype(np.float32)
    bhh = rng.uniform(-s, s, G4).astype(np.float32)
    h0 = (rng.standard_normal((B, H)) * 0.04).astype(np.float32)
    c0 = (rng.standard_normal((B, H)) * 0.04).astype(np.float32)

    # numpy reference
    bb = bih + bhh
    h, c = h0.copy(), c0.copy()
    hs_ref = np.empty((B, t_steps, H), np.float32)
    sig = lambda v: 1.0 / (1.0 + np.exp(-v))
    for t in range(t_steps):
        gates = x[:, t] @ Wih.T + h @ Whh.T + bb
        i, f, g, o = np.split(gates, 4, axis=1)
        c = sig(f) * c + sig(i) * np.tanh(g)
        h = sig(o) * np.tanh(c)
        hs_ref[:, t] = h

    nc = build_lstm(t_steps)
    in_maps = prepare_inputs(x, Wih, Whh, bih, bhh, h0, c0, t_steps)

    sim = CoreSim(nc)
    for name, val in in_maps[0].items():
        sim.tensor(name)[:] = val
    sim.simulate()
    hsT_out = np.asarray(sim.tensor("hsT"))
    hs = _unfold_hs(hsT_out, BSH)
    ref = hs_ref[:BSH]
    err = np.abs(hs - ref).max()
    rel = err / max(np.abs(ref).max(), 1e-9)
    print(f"sim t_steps={t_steps}: max abs err={err:.3e} rel={rel:.3e}")
    assert rel < 1e-4, "sim mismatch"
    print("SIM PASS")


if __name__ == "__main__":
    import sys
    _sim_check(int(sys.argv[1]) if len(sys.argv) > 1 else 16)
